# revision 1
# baseline (speedup 1.0000x reference)
"""LIF spike-train kernel for Trainium2 (Bass/Tile), data-parallel over 8 cores.

Reference semantics (T=4, tau=0.5, thresh=1.0), per element:
    mem = 0
    for t in range(4):
        mem = mem*0.5 + x[t]
        s[t] = (mem - 1 >= 0)
        mem = mem - s[t]

x: [T*B, C, H, W] = [256, 128, 32, 32] f32, viewed as [4, 64, 128, 1024].
Batch dim (64) is sharded 8-ways; each core streams [4, 8, 128, 1024].

Every step is bit-exact vs the reference in fp32: mult by 0.5/-1 is exact,
the compare (mem >= 1) <=> (mem - 1 >= 0), and the adds round identically
regardless of fusion.
"""

import os
import sys

sys.path.insert(0, "/opt/trn_rl_repo")

import numpy as np

T = 4
B = 64
C = 128
HW = 1024
NCORES = 8
BLOC = B // NCORES  # 8 batch elements per core

LAST_EXEC_NS = None
LAST_TRACE = None

_CACHE = {}


def _build(bloc=BLOC):
    """Build the per-core Bass module.

    The computation is purely elementwise within each timestep, so the
    partition mapping is arbitrary. Viewing each t-block [bloc, C, HW] as a
    flat [128, F] (F = bloc*C*HW/128) gives F*4-byte contiguous DRAM runs
    per partition -> large DMA descriptors -> near-peak HBM bandwidth.
    x, y: [T, 128, F] f32.
    """
    import concourse.bacc as bacc
    import concourse.mybir as mybir
    from concourse import tile

    f32 = mybir.dt.float32
    mult = mybir.AluOpType.mult
    add = mybir.AluOpType.add
    is_ge = mybir.AluOpType.is_ge

    F = bloc * C * HW // 128  # flat free width per t-block (8192 for bloc=8)
    W = min(int(os.environ.get("LIF_W", "2048")), F)  # chunk width
    NCH = F // W
    assert F % W == 0

    nc = bacc.Bacc("TRN2", target_bir_lowering=False, debug=False, num_devices=NCORES)
    x = nc.dram_tensor("x", [T, 128, F], f32, kind="ExternalInput").ap()
    y = nc.dram_tensor("y", [T, 128, F], f32, kind="ExternalOutput").ap()

    from concourse.vector_clock import ScopedClock

    class _SlimTailTile(tile.TileContext):
        """One-shot kernel tail: keep the drain (stores must land) and the
        semaphore clears (NEFF may be executed repeatedly), but use a
        sem-only first barrier and drop the trailing all-engine barrier —
        NEFF completion already waits for every engine's stream end."""

        def _drain_and_barrier(self, tick_clock, wait_clock):
            drain_inst = self.nc.sync.drain()
            wait_clock.add_sem_waits(
                drain_inst.ins, ScopedClock({None: tick_clock.global_clock})
            )
            self.nc.all_engine_barrier(sem_only=True)
            assert self.sems is not None
            popped = self.nc._tile_sem_poison_stack.pop()
            assert popped is self._sem_poison
            self.nc.clear_and_free_semaphores(
                list(self.sems.allocated().values())
            )

    # Measured A/B: the slim tail is slower and noisier (median 109us vs
    # 100us) — the standard drain+barrier tail gives a faster completion
    # path. Keep the stock tail; slim available via env for experiments.
    tile_cls = (
        _SlimTailTile if os.environ.get("LIF_SLIMTAIL") == "1" else tile.TileContext
    )
    # Measured A/B: x bufs=6 with u trimmed to 4 (same SBUF total) beats the
    # uniform bufs=5 by ~4us median — deeper input prefetch rides through
    # HBM-contention stalls.
    xbufs = int(os.environ.get("LIF_XBUFS", "6"))
    with tile_cls(nc) as tc:
        with tc.tile_pool(name="p", bufs=xbufs) as pool:
            ringsplit = os.environ.get("LIF_RINGSPLIT") == "1"
            vs = {}
            for t in range(T):
                xs = {}
                for i in range(NCH):
                    xt = pool.tile([128, W], f32, tag="x")
                    ld = nc.scalar if (ringsplit and i % 2) else nc.sync
                    ld.dma_start(out=xt, in_=x[t][:, i * W : (i + 1) * W])
                    xs[i] = xt

                if t == 0:
                    # mem = x0; s = (mem >= 1); v = mem - s
                    us = xs
                else:
                    us = {}
                    for i in range(NCH):
                        # u = 0.5*v + x[t]
                        u = pool.tile(
                            [128, W],
                            f32,
                            tag="u",
                            bufs=int(os.environ.get("LIF_UBUFS", "4")),
                        )
                        nc.vector.scalar_tensor_tensor(
                            u, vs[i], 0.5, xs[i], mult, add
                        )
                        us[i] = u

                ss = {}
                tailsplit = os.environ.get("LIF_TAILSPLIT") == "1"
                for i in range(NCH):
                    # s = (u >= 1)
                    st = pool.tile(
                        [128, W],
                        f32,
                        tag="s",
                        bufs=int(os.environ.get("LIF_SBUFS", "5")),
                    )
                    if tailsplit and t == T - 1 and i == NCH - 1:
                        # Final chunk: halve compare+store so the last store
                        # overlaps the last compare instead of serializing.
                        h = W // 2
                        nc.vector.tensor_scalar(
                            st[:, :h], us[i][:, :h], 1.0, None, is_ge
                        )
                        nc.vector.tensor_scalar(
                            st[:, h:], us[i][:, h:], 1.0, None, is_ge
                        )
                    else:
                        nc.vector.tensor_scalar(st, us[i], 1.0, None, is_ge)
                    ss[i] = st
                if t < T - 1:
                    # All compute on DVE: GpSimd shares SBUF ports with the
                    # vector engine and its contention knocks tensor_scalar
                    # out of its 2x perf mode (measured 631 -> 2988 ns).
                    sub_eng = (
                        nc.gpsimd
                        if os.environ.get("LIF_GPSIMD") == "1"
                        else nc.vector
                    )
                    for i in range(NCH):
                        # v = u - s  (next membrane, post soft-reset)
                        v = pool.tile([128, W], f32, tag=f"v{i}", bufs=2)
                        sub_eng.tensor_sub(v, us[i], ss[i])
                        vs[i] = v
                for i in range(NCH):
                    st_eng = nc.sync if (ringsplit and i % 2) else nc.scalar
                    if tailsplit and t == T - 1 and i == NCH - 1:
                        h = W // 2
                        base = i * W
                        st_eng.dma_start(
                            out=y[t][:, base : base + h], in_=ss[i][:, :h]
                        )
                        st_eng.dma_start(
                            out=y[t][:, base + h : base + W], in_=ss[i][:, h:]
                        )
                    else:
                        st_eng.dma_start(
                            out=y[t][:, i * W : (i + 1) * W], in_=ss[i]
                        )

    nc.compile()
    return nc


def _build_raw(bloc=BLOC):
    """Raw bacc version: hand-rolled semaphores, no Tile end-of-kernel
    drain/barrier. Same math and flat [T, 128, F] layout as _build()."""
    import concourse.bacc as bacc
    import concourse.mybir as mybir

    f32 = mybir.dt.float32
    mult = mybir.AluOpType.mult
    add = mybir.AluOpType.add
    is_ge = mybir.AluOpType.is_ge

    F = bloc * C * HW // 128
    W = min(int(os.environ.get("LIF_W", "2048")), F)
    NCH = F // W
    assert F % W == 0
    NX = 2 * NCH  # x ring slots (two t-phases deep)
    NS = NCH  # s ring slots
    NIDX = T * NCH

    nc = bacc.Bacc("TRN2", target_bir_lowering=False, debug=False, num_devices=NCORES)
    x = nc.dram_tensor("x", [T, 128, F], f32, kind="ExternalInput").ap()
    y = nc.dram_tensor("y", [T, 128, F], f32, kind="ExternalOutput").ap()

    X = [nc.alloc_sbuf_tensor(f"X{k}", [128, W], f32).ap() for k in range(NX)]
    S = [nc.alloc_sbuf_tensor(f"S{k}", [128, W], f32).ap() for k in range(NS)]
    U = [nc.alloc_sbuf_tensor(f"U{k}", [128, W], f32).ap() for k in range(2)]
    V = [
        [nc.alloc_sbuf_tensor(f"V{c}_{k}", [128, W], f32).ap() for k in range(2)]
        for c in range(NCH)
    ]

    from contextlib import ExitStack

    with ExitStack() as stack:
        block = stack.enter_context(nc.Block())
        xf = stack.enter_context(nc.semaphore("xf"))  # DVE consumed an x tile
        sr = stack.enter_context(nc.semaphore("sr"))  # DVE produced an s tile
        xs = [stack.enter_context(nc.semaphore(f"xs{k}")) for k in range(NX)]
        ss = [stack.enter_context(nc.semaphore(f"ss{k}")) for k in range(NS)]

        @block.sync
        def _(sp: object):
            for idx in range(NIDX):
                t, c = idx // NCH, idx % NCH
                slot = idx % NX
                if idx >= NX:
                    sp.wait_ge(xf, idx - NX + 1)
                sp.dma_start(
                    out=X[slot], in_=x[t][:, c * W : (c + 1) * W]
                ).then_inc(xs[slot], 16)

        @block.vector
        def _(ve: object):
            for idx in range(NIDX):
                t, c = idx // NCH, idx % NCH
                slot = idx % NX
                sslot = idx % NS
                ve.wait_ge(xs[slot], 16 * (idx // NX + 1))
                if idx >= NS:
                    ve.wait_ge(ss[sslot], 16 * (idx // NS))
                if t == 0:
                    ve.tensor_scalar(S[sslot], X[slot], 1.0, None, is_ge).then_inc(
                        sr, 1
                    )
                    ve.drain()
                    ve.tensor_sub(V[c][0], X[slot], S[sslot]).then_inc(xf, 1)
                    ve.drain()
                else:
                    u = U[idx % 2]
                    ve.scalar_tensor_tensor(
                        u, V[c][(t - 1) % 2], 0.5, X[slot], mult, add
                    ).then_inc(xf, 1)
                    ve.drain()
                    ve.tensor_scalar(S[sslot], u, 1.0, None, is_ge).then_inc(sr, 1)
                    ve.drain()
                    if t < T - 1:
                        ve.tensor_sub(V[c][t % 2], u, S[sslot])
                        ve.drain()

        @block.scalar
        def _(act: object):
            for idx in range(NIDX):
                t, c = idx // NCH, idx % NCH
                sslot = idx % NS
                act.wait_ge(sr, idx + 1)
                act.dma_start(
                    out=y[t][:, c * W : (c + 1) * W], in_=S[sslot]
                ).then_inc(ss[sslot], 16)
            for k in range(NS):
                act.wait_ge(ss[k], 16 * (NIDX // NS))

    nc.compile()
    return nc


def _get_nc():
    if "nc" not in _CACHE:
        builder = _build_raw if os.environ.get("LIF_RAW") == "1" else _build
        _CACHE["nc"] = builder()
    return _CACHE["nc"]


def kernel(x: np.ndarray) -> np.ndarray:
    global LAST_EXEC_NS, LAST_TRACE
    from concourse.bass_utils import run_bass_kernel_spmd

    x = np.ascontiguousarray(np.asarray(x), dtype=np.float32)
    assert x.shape == (T * B, C, 32, 32), x.shape
    xv = x.reshape(T, B, C, HW)

    F = BLOC * C * HW // 128
    in_maps = []
    for m in range(NCORES):
        shard = np.ascontiguousarray(xv[:, m * BLOC : (m + 1) * BLOC]).reshape(
            T, 128, F
        )
        in_maps.append({"x": shard})

    nc = _get_nc()
    trace = os.environ.get("LIF_TRACE") == "1"
    res = run_bass_kernel_spmd(nc, in_maps, core_ids=list(range(NCORES)), trace=trace)
    LAST_EXEC_NS = res.exec_time_ns
    if res.instructions_and_trace is not None:
        LAST_TRACE = res.instructions_and_trace[1]

    out = np.empty((T, B, C, HW), dtype=np.float32)
    for m in range(NCORES):
        out[:, m * BLOC : (m + 1) * BLOC] = res.results[m]["y"].reshape(
            T, BLOC, C, HW
        )
    return out.reshape(T * B, C, 32, 32)


def _sim_in_out_shape(bloc):
    return (T, 128, bloc * C * HW // 128)



# revision 2
# speedup vs baseline: 1.1004x; 1.1004x over previous
"""LIF spike-train kernel for Trainium2 (Bass/Tile), data-parallel over 8 cores.

Reference semantics (T=4, tau=0.5, thresh=1.0), per element:
    mem = 0
    for t in range(4):
        mem = mem*0.5 + x[t]
        s[t] = (mem - 1 >= 0)
        mem = mem - s[t]

x: [T*B, C, H, W] = [256, 128, 32, 32] f32, viewed as [4, 64, 128, 1024].
Batch dim (64) is sharded 8-ways; each core streams [4, 8, 128, 1024].

Key traffic optimization: spikes are exactly 0.0/1.0, so the on-device
output tensor is uint8 (is_ge writes the u8 tile directly, the host casts
back to f32 during unshard). That cuts the store stream from 16.78 MB to
4.19 MB per core; with the mandatory 16.78 MB f32 input read the HBM
floor drops from ~94us to ~59us per core at 358 GB/s.

Every step is bit-exact vs the reference in fp32: mult by 0.5/-1 is exact,
the compare (mem >= 1) <=> (mem - 1 >= 0), and the adds round identically
regardless of fusion. The u8 spike re-enters the recurrence via a
mixed-dtype subtract (f32 - u8) which converts 0/1 exactly.
"""

import os
import sys

sys.path.insert(0, "/opt/trn_rl_repo")

import numpy as np

T = 4
B = 64
C = 128
HW = 1024
NCORES = 8
BLOC = B // NCORES  # 8 batch elements per core

LAST_EXEC_NS = None
LAST_TRACE = None

_CACHE = {}


def _build(bloc=BLOC):
    """Build the per-core Bass module.

    The computation is purely elementwise within each timestep, so the
    partition mapping is arbitrary. Viewing each t-block [bloc, C, HW] as a
    flat [128, F] (F = bloc*C*HW/128) gives F*4-byte contiguous DRAM runs
    per partition -> large DMA descriptors -> near-peak HBM bandwidth.
    x: [T, 128, F] f32, y: [T, 128, F] u8.
    """
    import concourse.bacc as bacc
    import concourse.mybir as mybir
    from concourse import tile

    f32 = mybir.dt.float32
    u8 = mybir.dt.uint8
    mult = mybir.AluOpType.mult
    add = mybir.AluOpType.add
    is_ge = mybir.AluOpType.is_ge

    odt = f32 if os.environ.get("LIF_OUT") == "f32" else u8

    F = bloc * C * HW // 128  # flat free width per t-block (8192 for bloc=8)
    W = min(int(os.environ.get("LIF_W", "2048")), F)  # chunk width
    NCH = F // W
    assert F % W == 0

    nc = bacc.Bacc("TRN2", target_bir_lowering=False, debug=False, num_devices=NCORES)
    x = nc.dram_tensor("x", [T, 128, F], f32, kind="ExternalInput").ap()
    y = nc.dram_tensor("y", [T, 128, F], odt, kind="ExternalOutput").ap()

    # One [128, F] u8 spike tile per t (stored whole: 1 MiB descriptors)
    # vs per-chunk [128, W] tiles (stored per chunk). A/B via LIF_STORE_T.
    store_t = os.environ.get("LIF_STORE_T", "1") == "1"

    xbufs = int(os.environ.get("LIF_XBUFS", "6"))
    ubufs = int(os.environ.get("LIF_UBUFS", "4"))
    sbufs = int(os.environ.get("LIF_SBUFS", "3"))
    with tile.TileContext(nc) as tc:
        with tc.tile_pool(name="p", bufs=xbufs) as pool:
            vs = {}
            for t in range(T):
                xs = {}
                for i in range(NCH):
                    xt = pool.tile([128, W], f32, tag="x")
                    nc.sync.dma_start(out=xt, in_=x[t][:, i * W : (i + 1) * W])
                    xs[i] = xt

                if t == 0:
                    # mem = x0; s = (mem >= 1); v = mem - s
                    us = xs
                else:
                    us = {}
                    for i in range(NCH):
                        # u = 0.5*v + x[t]
                        u = pool.tile([128, W], f32, tag="u", bufs=ubufs)
                        nc.vector.scalar_tensor_tensor(
                            u, vs[i], 0.5, xs[i], mult, add
                        )
                        us[i] = u

                ss = {}
                if store_t:
                    s_t = pool.tile([128, F], odt, tag="s", bufs=sbufs)
                for i in range(NCH):
                    # s = (u >= 1), written directly as u8 0/1
                    if store_t:
                        st = s_t[:, i * W : (i + 1) * W]
                    else:
                        st = pool.tile([128, W], odt, tag="s", bufs=sbufs + 2)
                    nc.vector.tensor_scalar(st, us[i], 1.0, None, is_ge)
                    ss[i] = st
                if t < T - 1:
                    for i in range(NCH):
                        # v = u - s  (next membrane, post soft-reset);
                        # mixed-dtype read of the u8 spike converts exactly.
                        v = pool.tile([128, W], f32, tag=f"v{i}", bufs=2)
                        nc.vector.tensor_sub(v, us[i], ss[i])
                        vs[i] = v
                if store_t:
                    nc.scalar.dma_start(out=y[t], in_=s_t)
                else:
                    for i in range(NCH):
                        nc.scalar.dma_start(
                            out=y[t][:, i * W : (i + 1) * W], in_=ss[i]
                        )

    nc.compile()
    return nc


def _get_nc():
    if "nc" not in _CACHE:
        _CACHE["nc"] = _build()
    return _CACHE["nc"]


def kernel(x: np.ndarray) -> np.ndarray:
    global LAST_EXEC_NS, LAST_TRACE
    from concourse.bass_utils import run_bass_kernel_spmd

    x = np.ascontiguousarray(np.asarray(x), dtype=np.float32)
    assert x.shape == (T * B, C, 32, 32), x.shape
    xv = x.reshape(T, B, C, HW)

    F = BLOC * C * HW // 128
    in_maps = []
    for m in range(NCORES):
        shard = np.ascontiguousarray(xv[:, m * BLOC : (m + 1) * BLOC]).reshape(
            T, 128, F
        )
        in_maps.append({"x": shard})

    nc = _get_nc()
    trace = os.environ.get("LIF_TRACE") == "1"
    res = run_bass_kernel_spmd(nc, in_maps, core_ids=list(range(NCORES)), trace=trace)
    LAST_EXEC_NS = res.exec_time_ns
    if res.instructions_and_trace is not None:
        LAST_TRACE = res.instructions_and_trace[1]

    out = np.empty((T, B, C, HW), dtype=np.float32)
    for m in range(NCORES):
        # u8 (or f32) shard -> f32 full output; numpy casts 0/1 exactly.
        out[:, m * BLOC : (m + 1) * BLOC] = res.results[m]["y"].reshape(
            T, BLOC, C, HW
        )
    return out.reshape(T * B, C, 32, 32)


def _sim_in_out_shape(bloc):
    return (T, 128, bloc * C * HW // 128)
